# revision 13
# baseline (speedup 1.0000x reference)
"""CenterOfMassLoss Trainium2 kernel.

Layout / strategy
-----------------
Inputs: predicted, target [1, 31, 2048, 2048] f32.
9 regions = 3 row-bands x 3 col-bands, each 400x400, bands start at
{200, 1000, 1500}. Loss needs, per (channel, region):
  - S  = sum x^3, Sx = sum (h-199.5) x^3, Sy = sum (w-199.5) x^3  for both
    tensors (center of mass, centered coordinates), and
  - R  = sum target (raw) over the region,
plus the global mean of target.

Sharding: channels across 8 cores (4/4/4/4/4/4/4/3+dup).  Each core, per
channel:
  * target: stream the full image as 8 [128, 2, 2048] tiles (2 MiB contiguous
    DMAs).  A [128,4]-stationary tf32 (float32r) matmul per 512-col chunk
    accumulates PSUM R[4, 2048] = per-column sums for {all rows, band0 rows,
    band1 rows, band2 rows}.  Band sub-tiles also get x^3 (ACT square + DVE
    mul, cols [200,1900)) and a [128,12]-stationary tf32 matmul accumulating
    {band x tensor x (S, (h-199.5)S)} into PSUM P_j[12, 400] per col-band.
  * predicted: packed on host to regions-only, pre-swizzled [128, 10, 1200]
    (partition p, row-tile t, col w <-> packed row 128t+p), streamed as
    5 [128, 2, 1200] tiles through the same cube+moment path.
float32r (tf32) matmuls run 4x faster than float32 on the PE; the target is
pre-rounded to the tf32 grid on host (the BIR verifier requires f32r matmul
operands to be produced rounded), cube outputs are rounded by the DVE on
write, and the CoM weights are exactly representable.  Per-core outputs are
tiny (moments [4,3,12,400], rawsums [4,4,2048]); the final ~1k-flop
combination runs on host in float64.
"""

import numpy as np

# ---------------- problem constants (hardcoded) ----------------
N_CORES = 8
CHANNELS = 31
H = W = 2048
NCH = 4  # channel slots per core
BS = [200, 1000, 1500]  # band starts (rows and cols)
RS = 400  # region side
NT_T = 16  # target row tiles of 128 (processed 2 per DMA)
NT_P = 10  # pred row tiles (9 x 128 + 48 rows + zero pad)
PRED_N = 3 * RS  # 1200
SPAN0, SPAN1 = 200, 1900  # cube span covering all 3 col bands
SPW = SPAN1 - SPAN0  # 1700
CENTER = (RS - 1) / 2.0  # 199.5
FUNDAMENTAL_INDEX = 4
FUNDA_WEIGHT = 5.0

# channel assignment per core: 7 cores x 4 channels + core 7 [28,29,30,30(dup)]
ASSIGN = [list(range(4 * k, 4 * k + 4)) for k in range(7)] + [[28, 29, 30, 30]]
VALID_SLOTS = [4, 4, 4, 4, 4, 4, 4, 3]  # dup slot ignored on host


def _band_of_row(g):
    """absolute image row -> (band, h within band) or None"""
    for b, s in enumerate(BS):
        if s <= g < s + RS:
            return b, g - s
    return None


def make_weights():
    """Stationary matrices for the PE reductions (host-computed constants).

    wraw [128, 16*4]:  per target tile t, cols (4t..4t+4) =
        [ones, band0 mask, band1 mask, band2 mask] for partition row 128t+p.
    wct  [128, 16*12]: per target tile t, 12 cols = 3 bands x
        [predS, predSx, targS, targSx]; target fills cols 4b+2, 4b+3.
    wcp  [128, 10*12]: per pred tile t (packed row 128t+p), pred fills
        cols 4b+0, 4b+1.  All values exactly representable in tf32.
    """
    wraw = np.zeros((128, NT_T * 4), dtype=np.float32)
    wct = np.zeros((128, NT_T * 12), dtype=np.float32)
    for t in range(NT_T):
        for p in range(128):
            g = 128 * t + p
            wraw[p, 4 * t + 0] = 1.0
            bh = _band_of_row(g)
            if bh is not None:
                b, h = bh
                wraw[p, 4 * t + 1 + b] = 1.0
                wct[p, 12 * t + 4 * b + 2] = 1.0
                wct[p, 12 * t + 4 * b + 3] = h - CENTER
    wcp = np.zeros((128, NT_P * 12), dtype=np.float32)
    for t in range(NT_P):
        for p in range(128):
            g = 128 * t + p
            if g < PRED_N:
                b, h = g // RS, g % RS
                wcp[p, 12 * t + 4 * b + 0] = 1.0
                wcp[p, 12 * t + 4 * b + 1] = h - CENTER
    return wraw, wct, wcp


def round_tf32(a):
    """Round float32 array to the tf32 grid (10 mantissa bits, RNE)."""
    u = a.view(np.uint32) if a.flags['C_CONTIGUOUS'] else \
        np.ascontiguousarray(a).view(np.uint32)
    u = u.astype(np.uint64)
    u = (u + 0xFFF + ((u >> 13) & 1)) & 0xFFFFE000
    return u.astype(np.uint32).view(np.float32)


# target tiles that intersect a region row-band
BAND_TILES = [t for t in range(NT_T)
              if any(_band_of_row(128 * t + p) for p in range(128))]
LAST_BAND_TILE = BAND_TILES[-1]
# per-channel target processing order (see build_nc): full-band DMA pairs
# first, mixed band/non-band pairs after, tiles 14/15 as singles last
PAIR_ORDER = [1, 4, 5, 6, 0, 2, 3]
SINGLE_TILES = [14, 15]
FIRST_RAW_TILE = 2 * PAIR_ORDER[0]


def build_nc():
    """Build the per-core Bass program (same program on all 8 cores)."""
    import concourse.bacc as bacc
    import concourse.tile as tile
    from concourse import mybir

    F32 = mybir.dt.float32
    F32R = mybir.dt.float32r  # tf32: PE streams 1 cycle/row (fp32 takes 4)
    nc = bacc.Bacc("TRN2", debug=False)

    targ = nc.dram_tensor("targ", [NCH, H, W], F32R, kind="ExternalInput")
    pred = nc.dram_tensor("pred", [NCH, 128, NT_P, PRED_N], F32,
                          kind="ExternalInput")
    wraw_d = nc.dram_tensor("wraw", [128, NT_T * 4], F32R, kind="ExternalInput")
    wct_d = nc.dram_tensor("wct", [128, NT_T * 12], F32R, kind="ExternalInput")
    wcp_d = nc.dram_tensor("wcp", [128, NT_P * 12], F32R, kind="ExternalInput")
    mom_out = nc.dram_tensor("moments", [NCH, 3, 12, RS], F32, kind="ExternalOutput")
    raw_out = nc.dram_tensor("rawsums", [NCH, 4, W], F32, kind="ExternalOutput")

    with tile.TileContext(nc) as tc:
        with (
            tc.tile_pool(name="consts", bufs=1) as consts,
            tc.tile_pool(name="tpool", bufs=5) as tpool,
            tc.tile_pool(name="ppool", bufs=3) as ppool,
            tc.tile_pool(name="sqt", bufs=3) as sqtp,
            tc.tile_pool(name="cbt", bufs=3) as cbtp,
            tc.tile_pool(name="sqp", bufs=2) as sqpp,
            tc.tile_pool(name="cbp", bufs=2) as cbpp,
            tc.tile_pool(name="outsb", bufs=2) as outsb,
            tc.tile_pool(name="psum", bufs=1, space="PSUM") as psum,
        ):
            # weight loads ride the ACT HWDGE ring so the first big target
            # loads start immediately on the SP ring
            wraw_sb = consts.tile([128, NT_T * 4], F32R)
            nc.scalar.dma_start(out=wraw_sb[:], in_=wraw_d[:])
            wct_sb = consts.tile([128, NT_T * 12], F32R)
            nc.scalar.dma_start(out=wct_sb[:], in_=wct_d[:])
            wcp_sb = consts.tile([128, NT_P * 12], F32R)
            nc.scalar.dma_start(out=wcp_sb[:], in_=wcp_d[:])

            for ci in range(NCH):
                r_ps = psum.tile([4, W], F32, tag="r_ps")
                p_ps = [
                    psum.tile([12, RS], F32, tag=f"p_ps{j}", name=f"p_ps{j}")
                    for j in range(3)
                ]

                # ---- predicted (packed regions, pre-swizzled) ----
                # u=0..3: [128, 2, 1200] pairs; then t=8 [128,1200] and the
                # 48-row remainder t=9 (skips transferring the zero pad)
                for u in range(4):
                    ptile = ppool.tile([128, 2, PRED_N], F32, tag="ptile")
                    nc.sync.dma_start(
                        out=ptile[:], in_=pred[ci, :, 2 * u:2 * u + 2, :]
                    )
                    psq = sqpp.tile([128, 2, PRED_N], F32, tag="psq")
                    nc.scalar.square(psq[:], ptile[:])
                    pcb = cbpp.tile([128, 2, PRED_N], F32R, tag="pcb")
                    nc.vector.tensor_mul(pcb[:], psq[:], ptile[:])
                    for i in range(2):
                        t = 2 * u + i
                        for j in range(3):
                            nc.tensor.matmul(
                                p_ps[j][:, :],
                                wcp_sb[:, 12 * t:12 * t + 12],
                                pcb[:, i, RS * j:RS * (j + 1)],
                                start=(t == 0),
                                stop=False,
                            )
                for t, kk in ((8, 128), (9, PRED_N - 128 * (NT_P - 1))):
                    ptile = ppool.tile([128, 2, PRED_N], F32, tag="ptile")
                    nc.sync.dma_start(
                        out=ptile[:kk, 0, :], in_=pred[ci, :kk, t, :]
                    )
                    psq = sqpp.tile([128, 2, PRED_N], F32, tag="psq")
                    nc.scalar.square(psq[:kk, 0, :], ptile[:kk, 0, :])
                    pcb = cbpp.tile([128, 2, PRED_N], F32R, tag="pcb")
                    nc.vector.tensor_mul(
                        pcb[:kk, 0, :], psq[:kk, 0, :], ptile[:kk, 0, :]
                    )
                    for j in range(3):
                        nc.tensor.matmul(
                            p_ps[j][:, :],
                            wcp_sb[:kk, 12 * t:12 * t + 12],
                            pcb[:kk, 0, RS * j:RS * (j + 1)],
                            start=False,
                            stop=False,
                        )

                # ---- target (full image, 2 row-tiles per DMA) ----
                # Order: full-band pairs, then mixed pairs, then t14/t15 as
                # single-tile DMAs — so only cheap raw matmuls trail the
                # final DMA of the channel.
                def do_tile(t, tt_ap):
                    for n in range(4):
                        nc.tensor.matmul(
                            r_ps[:, 512 * n:512 * (n + 1)],
                            wraw_sb[:, 4 * t:4 * t + 4],
                            tt_ap[:, 512 * n:512 * (n + 1)],
                            start=(t == FIRST_RAW_TILE),
                            stop=(t == NT_T - 1),
                        )
                    if t in BAND_TILES:
                        span = tt_ap[:, SPAN0:SPAN1].bitcast(F32)
                        tsq = sqtp.tile([128, SPW], F32, tag="tsq", name="tsq")
                        nc.scalar.square(tsq[:], span)
                        tcb = cbtp.tile([128, SPW], F32R, tag="tcb", name="tcb")
                        nc.vector.tensor_mul(tcb[:], tsq[:], span)
                        for j in range(3):
                            nc.tensor.matmul(
                                p_ps[j][:, :],
                                wct_sb[:, 12 * t:12 * t + 12],
                                tcb[:, BS[j] - SPAN0:BS[j] - SPAN0 + RS],
                                start=False,
                                stop=(t == LAST_BAND_TILE),
                            )

                for u in PAIR_ORDER:
                    ttile = tpool.tile([128, 2, W], F32R, tag="ttile")
                    nc.sync.dma_start(
                        out=ttile[:],
                        in_=targ[ci, 256 * u:256 * (u + 1), :].rearrange(
                            "(i p) w -> p i w", p=128
                        ),
                    )
                    for i in range(2):
                        do_tile(2 * u + i, ttile[:, i, :])
                for t in SINGLE_TILES:
                    tsing = tpool.tile([128, 2, W], F32R, tag="ttile",
                                       name="tsing")
                    nc.sync.dma_start(
                        out=tsing[:, 0, :], in_=targ[ci, 128 * t:128 * (t + 1), :]
                    )
                    do_tile(t, tsing[:, 0, :])

                # ---- evacuate PSUM -> SBUF -> DRAM ----
                for j in range(3):
                    mout = outsb.tile([12, RS], F32, tag="mout")
                    nc.scalar.copy(mout[:], p_ps[j][:])
                    nc.sync.dma_start(out=mom_out[ci, j], in_=mout[:])
                rout = outsb.tile([4, W], F32, tag="rout")
                nc.scalar.copy(rout[:], r_ps[:])
                nc.sync.dma_start(out=raw_out[ci], in_=rout[:])

    nc.compile()
    return nc


_NC = None


def _get_nc():
    global _NC
    if _NC is None:
        _NC = build_nc()
    return _NC


def pack_pred(p3, chs):
    """[31,H,W] -> [NCH, 128, NT_P, PRED_N] region-packed + row-swizzled."""
    pc = np.zeros((NCH, 128, NT_P, PRED_N), dtype=np.float32)
    rows = np.empty((PRED_N, PRED_N), dtype=np.float32)
    for s, ch in enumerate(chs):
        for b in range(3):
            for j in range(3):
                rows[RS * b:RS * (b + 1), RS * j:RS * (j + 1)] = \
                    p3[ch, BS[b]:BS[b] + RS, BS[j]:BS[j] + RS]
        # packed row g = 128*t + p  ->  pc[s, p, t, :]
        full = rows[:128 * (NT_P - 1)].reshape(NT_P - 1, 128, PRED_N)
        pc[s, :, :NT_P - 1, :] = full.transpose(1, 0, 2)
        rem = PRED_N - 128 * (NT_P - 1)  # 48
        pc[s, :rem, NT_P - 1, :] = rows[128 * (NT_P - 1):]
    return pc


def make_in_maps(predicted, target):
    """Pack full inputs into per-core in_maps."""
    predicted = np.asarray(predicted, dtype=np.float32)
    target = np.asarray(target, dtype=np.float32)
    p3 = predicted[0]  # [31, H, W]
    t3 = round_tf32(np.ascontiguousarray(target[0]))  # tf32 grid for f32r MMs
    wraw, wct, wcp = make_weights()
    in_maps = []
    for k in range(N_CORES):
        chs = ASSIGN[k]
        tc = np.ascontiguousarray(t3[chs])  # [4, H, W]
        pc = pack_pred(p3, chs)
        in_maps.append(
            {"targ": tc, "pred": pc, "wraw": wraw, "wct": wct, "wcp": wcp}
        )
    return in_maps


def combine(results):
    """Host-side final math (float64) from per-core outputs."""
    iw = np.arange(RS, dtype=np.float64) - CENTER
    norms = np.zeros((9, CHANNELS), dtype=np.float64)
    rraw = np.zeros((9, CHANNELS), dtype=np.float64)
    gsum = 0.0
    for k in range(N_CORES):
        mom = np.asarray(results[k]["moments"], dtype=np.float64)  # [4,3,12,400]
        raw = np.asarray(results[k]["rawsums"], dtype=np.float64)  # [4,4,W]
        for s in range(VALID_SLOTS[k]):
            ch = ASSIGN[k][s]
            gsum += raw[s, 0, :].sum()
            for b in range(3):
                for j in range(3):
                    reg = 3 * b + j
                    m = mom[s, j]
                    sp, sxp = m[4 * b + 0].sum(), m[4 * b + 1].sum()
                    syp = (m[4 * b + 0] * iw).sum()
                    st, sxt = m[4 * b + 2].sum(), m[4 * b + 3].sum()
                    syt = (m[4 * b + 2] * iw).sum()
                    dcx = sxp / sp - sxt / st
                    dcy = syp / sp - syt / st
                    norms[reg, ch] = np.sqrt(dcx * dcx + dcy * dcy)
                    rraw[reg, ch] = raw[s, 1 + b, BS[j]:BS[j] + RS].sum()
    mean_target = gsum / (CHANNELS * H * W)
    weighting = rraw / (RS * RS) / mean_target  # [9, 31]
    terms = (norms * weighting).sum(axis=1)  # [9]
    terms[FUNDAMENTAL_INDEX] *= FUNDA_WEIGHT
    total = terms.sum() / (CHANNELS * 9)
    return np.float32(total)


def kernel(predicted, target):
    from concourse.bass_utils import run_bass_kernel_spmd

    nc = _get_nc()
    in_maps = make_in_maps(predicted, target)
    res = run_bass_kernel_spmd(nc, in_maps, list(range(N_CORES)))
    return np.asarray(combine(res.results), dtype=np.float32)
